# revision 9
# baseline (speedup 1.0000x reference)
"""Banded (sliding-window) multi-head attention on 8 Trainium2 NeuronCores.

Problem: B=2, S=2048, D=512, H=8 heads (hd=64), window=256 (|i-j| <= 128),
  qkv = x @ Wqkv + bqkv           -> per-head q,k,v
  scores = (q k^T masked to band) / 8 ; softmax ; out = (attn v) @ Wo + bo

Sharding: core = (batch b in {0,1}) x (head-group g in {0..3}); each core
computes 2 heads over the full sequence of one batch element plus the o_proj
partial product for its heads' embed slice. The host sums the 4 partials per
batch and adds bo once.

Device-side layout (bf16 matmul operands, fp32 PSUM accumulation):
  - qkv projection computed TRANSPOSED: qkvT[fo, s], features permuted to
    [q0|q1|k0|k1|v0|v1] so Q^T/K^T/V^T per head live at partition offsets
    {0,64}.
  - scores key-major per 128-key block kb vs the 1-3 query blocks in band:
    ST[key, query] in PSUM -> ACT: P = exp(ST/8 + kmask[key]) -> DVE: the two
    128-col edge blocks *= triangular band mask (strided single op).
  - V^T re-transposed to natural [key, hd] via PE (bf16 transposes), with a
    ones column at 64/129 so denominators fall out of the AV matmul.
  - AV is FLIPPED vs v1: per query block qb, out[128q, 65] = sum_kb
    P[k, qb]^T @ [V|1]: 65-col matmuls (cheap in the col-based PE cost
    model), and the softmax denominator lands in PSUM as a *column*
    [128q, 1] -> strided reciprocal over 4 query blocks costs ~130ns,
    and no broadcast matmul is needed: normalization is a per-partition
    tensor_scalar multiply in SBUF bf16 (4x DVE mode).
  - valst_nat[q, d] is re-transposed per (h, qb) on PE into a per-chunk
    PSUM tile -> one copy -> d-major valstT for o_proj.
  - o_proj: outT[fo, s] partial = Wo_g^T @ valstT, written bf16 (host sums
    partials in fp32 and adds bo).
"""

import numpy as np
import ml_dtypes

import concourse.bass as bass  # noqa: F401
import concourse.mybir as mybir
import concourse.tile as tile
from concourse import bacc
from concourse.bass_utils import run_bass_kernel_spmd

B, S, DIN, E = 2, 2048, 512, 512
H, HD = 8, 64
NB = S // 128      # 16 key/query blocks of 128
NCHUNK = S // 512  # 4 query chunks of 512
F32 = mybir.dt.float32
F32R = mybir.dt.float32r
BF16 = mybir.dt.bfloat16
EXPF = mybir.ActivationFunctionType.Exp
IDENT = mybir.ActivationFunctionType.Identity
BF = ml_dtypes.bfloat16

_CACHE = {}
LAST_RESULTS = None  # BassKernelResults of the most recent run (for test.py)


def _build_nc():
    nc = bacc.Bacc(None, target_bir_lowering=False, debug=False)

    xt = nc.dram_tensor("xt", [4, DIN, 512], BF16, kind="ExternalInput")
    wq = nc.dram_tensor("wq", [128, 4, 384], BF16, kind="ExternalInput")
    wo = nc.dram_tensor("wo", [128, E], BF16, kind="ExternalInput")
    km = nc.dram_tensor("km", [128, NB], F32, kind="ExternalInput")
    tm = nc.dram_tensor("tm", [128, 256], BF16, kind="ExternalInput")
    idin = nc.dram_tensor("idin", [128, 128], BF16, kind="ExternalInput")
    outt = nc.dram_tensor("outt", [E, S], BF16, kind="ExternalOutput")

    with tile.TileContext(nc) as tc:
        with (
            tc.tile_pool(name="sb", bufs=1) as sb,
            tc.tile_pool(name="ps_qkv", bufs=2, space="PSUM") as ps_qkv,
            tc.tile_pool(name="ps_st", bufs=2, space="PSUM") as ps_st,
            tc.tile_pool(name="ps_av", bufs=2, space="PSUM") as ps_av,
            tc.tile_pool(name="ps_t", bufs=2, space="PSUM") as ps_t,
            tc.tile_pool(name="small", bufs=4) as small,
        ):
            xt_sb = sb.tile([128, 4, 4, 512], BF16)   # [p, kchunk, qchunk, q]
            wq_sb = sb.tile([128, 4, 384], BF16)      # [p, kchunk, fo]
            wo_sb = sb.tile([128, E], BF16)
            km_sb = sb.tile([128, NB], F32)
            tmE_sb = sb.tile([128, 2, 128], BF16)     # [lower | upper] edges
            qkvt = sb.tile([128, 3, S], BF16)         # fb0=Q, fb1=K, fb2=V
            vnat = sb.tile([128, NB, 130], BF16)      # [v0|1|v1|1] per key blk
            vln = sb.tile([128, 2, 4, 64], BF16)      # valst_nat per (h, qb)
            rc_sb = sb.tile([128, 2, 4], F32)         # 1/denom per (h, qb)
            valstT = sb.tile([128, S], BF16)          # d-major normalized AV
            outt_sb = sb.tile([128, 4, S], BF16)
            ident = sb.tile([128, 128], BF16)

            # weights/constants on the ACT HWDGE ring, xt on the SP ring;
            # both split by k-chunk so the first qkv group starts early
            for kc in range(4):
                nc.scalar.dma_start(out=wq_sb[:, kc, :], in_=wq[:, kc, :])
                nc.sync.dma_start(
                    out=xt_sb[:, kc, 0, :],
                    in_=xt[0, kc * 128:(kc + 1) * 128, :],
                )
            for cc in range(1, 4):
                nc.sync.dma_start(
                    out=xt_sb[:, :, cc, :],
                    in_=xt[cc].rearrange("(kc p) q -> p kc q", p=128),
                )
            nc.scalar.dma_start(out=km_sb, in_=km[:, :])
            nc.scalar.dma_start(
                out=tmE_sb, in_=tm.rearrange("p (a b) -> p a b", b=128))
            nc.sync.dma_start(out=ident, in_=idin[:, :])
            nc.sync.dma_start(out=wo_sb, in_=wo[:, :])

            # ones columns for the AV denominator
            nc.gpsimd.memset(vnat[:, :, 64:65], 1.0)
            nc.gpsimd.memset(vnat[:, :, 129:130], 1.0)

            # ---- qkv projection (transposed): qkvT = Wg^T @ x[b]^T ----
            def qkv_tile(cc, fb):
                ps = ps_qkv.tile([128, 512], F32, tag="qkv", name="ps")
                for kc in range(4):
                    nc.tensor.matmul(
                        ps,
                        wq_sb[:, kc, fb * 128:(fb + 1) * 128],
                        xt_sb[:, kc, cc, :],
                        start=(kc == 0),
                        stop=(kc == 3),
                    )
                dst = qkvt[:, fb, cc * 512:(cc + 1) * 512]
                if (cc, fb) in ((0, 0), (2, 0)):
                    nc.scalar.activation(out=dst, in_=ps, func=IDENT)
                else:
                    nc.vector.tensor_copy(dst, ps)

            # ---- V^T -> V natural [key, hd] with ones cols at 64/129 ----
            def pst_group(cc):
                kb0 = 4 * cc
                pst = ps_t.tile([128, 4, 128], BF16, tag="t", name="pst")
                for j in range(4):
                    kb = kb0 + j
                    nc.tensor.transpose(
                        pst[:, j, :], qkvt[:, 2, kb * 128:(kb + 1) * 128],
                        ident,
                    )
                nc.vector.tensor_copy(
                    vnat[:, kb0:kb0 + 4, :]
                    .rearrange("p k (g c) -> p k g c", c=65)[:, :, :, 0:64],
                    pst.rearrange("p k (g c) -> p k g c", c=64),
                )

            # ---- attention ----
            p_sb = [sb.tile([128, NB, 384], BF16, name=f"p{h}")
                    for h in range(2)]

            def scores_block(h, kb):
                hp = 64 * h
                ws, we = max(0, kb - 1), min(NB - 1, kb + 1)
                nq = (we - ws + 1) * 128
                st = ps_st.tile([128, 384], F32, tag="st", name="st")
                nc.tensor.matmul(
                    st[:, :nq],
                    qkvt[hp:hp + 64, 1, kb * 128:(kb + 1) * 128],
                    qkvt[hp:hp + 64, 0, ws * 128:(we + 1) * 128],
                    start=True,
                    stop=True,
                )
                nc.scalar.activation(
                    out=p_sb[h][:, kb, 0:nq],
                    in_=st[:, :nq],
                    func=EXPF,
                    bias=km_sb[:, kb:kb + 1],
                    scale=0.125,
                )
                # band-mask only the edge blocks (lower on the kb-1 block,
                # upper on the kb+1 block); the center block is all-valid.
                # Pool takes a share (SBUF-only op) to unload DVE.
                eng = nc.gpsimd if (2 * kb + h) % 8 < 3 else nc.vector
                pv = p_sb[h][:, kb, :].rearrange("p (a b) -> p a b", b=128)
                if kb == 0:
                    eng.tensor_mul(
                        pv[:, 1, :], pv[:, 1, :], tmE_sb[:, 1, :])
                elif kb == NB - 1:
                    eng.tensor_mul(
                        pv[:, 0, :], pv[:, 0, :], tmE_sb[:, 0, :])
                else:
                    eng.tensor_mul(
                        pv[:, 0::2, :], pv[:, 0::2, :], tmE_sb)

            # flipped AV: per query block, out[128q, 65] accumulates
            # P[k, qb]^T @ [V|1] over the 2-3 contributing key blocks
            av_tiles = {}  # h -> live av PSUM tile for the current chunk

            def av_qb(h, qb):
                if qb % 4 == 0 or h not in av_tiles:
                    av_tiles[h] = ps_av.tile(
                        [128, 4, 65], F32, tag="av", name="av")
                av = av_tiles[h]
                j = qb % 4
                kbs = [kb for kb in (qb - 1, qb, qb + 1) if 0 <= kb <= NB - 1]
                for i, kb in enumerate(kbs):
                    ws = max(0, kb - 1)
                    nc.tensor.matmul(
                        av[:, j, :],
                        p_sb[h][:, kb, (qb - ws) * 128:(qb - ws + 1) * 128],
                        vnat[:, kb, 65 * h:65 * h + 65],
                        start=(i == 0),
                        stop=(i == len(kbs) - 1),
                    )

            def chunk_finish(c):
                for h in range(2):
                    av = av_tiles.pop(h)
                    with nc.allow_low_precision("f32r softmax denom recip"):
                        nc.vector.reciprocal(rc_sb[:, h, :], av[:, :, 64])
                    nc.vector.tensor_copy(vln[:, h, :, :], av[:, :, 0:64])
                    for j in range(4):
                        nc.gpsimd.tensor_scalar_mul(
                            out=vln[:, h, j, :],
                            in0=vln[:, h, j, :],
                            scalar1=rc_sb[:, h, j:j + 1],
                        )
                # transpose normalized [q, d] -> d-major [2h*64, 512q]
                vt = ps_t.tile([128, 512], BF16, tag="t", name="vt")
                for h in range(2):
                    for j in range(4):
                        nc.tensor.transpose(
                            vt[64 * h:64 * h + 64, j * 128:(j + 1) * 128],
                            vln[:, h, j, :],
                            ident,
                        )
                nc.vector.tensor_copy(valstT[:, c * 512:(c + 1) * 512], vt)
                for fo in range(4):
                    po = ps_qkv.tile([128, 512], F32, tag="qkv", name="po")
                    nc.tensor.matmul(
                        po,
                        wo_sb[:, fo * 128:(fo + 1) * 128],
                        valstT[:, c * 512:(c + 1) * 512],
                        start=True,
                        stop=True,
                    )
                    dst = outt_sb[:, fo, c * 512:(c + 1) * 512]
                    if fo % 2 == 0:
                        nc.scalar.activation(out=dst, in_=po, func=IDENT)
                    else:
                        nc.vector.tensor_copy(dst, po)
                    nc.sync.dma_start(
                        out=outt[fo * 128:(fo + 1) * 128,
                                 c * 512:(c + 1) * 512],
                        in_=dst,
                    )

            # software pipeline: chunk-0 qkv + V up front, then per key
            # block: late qkv tiles for the next chunk, scores, the AV for
            # the query block completed by this kb, and the chunk epilogue
            # (normalize/transpose/o_proj) at chunk boundaries.
            for fb in range(3):
                qkv_tile(0, fb)
            pst_group(0)
            for kb in range(NB):
                cc = kb // 4
                if cc < 3:
                    if kb % 4 == 1:
                        qkv_tile(cc + 1, 0)
                    elif kb % 4 == 2:
                        qkv_tile(cc + 1, 1)
                    elif kb % 4 == 3:
                        qkv_tile(cc + 1, 2)
                        pst_group(cc + 1)
                for h in range(2):
                    scores_block(h, kb)
                if kb >= 1:
                    for h in range(2):
                        av_qb(h, kb - 1)
                    if kb % 4 == 0:
                        chunk_finish(kb // 4 - 1)
            for h in range(2):
                av_qb(h, NB - 1)
            chunk_finish(NCHUNK - 1)

    nc.finalize()
    return nc


def _numpy_reference(x, padding_mask, Wqkv, bqkv, Wo, bo):
    """Fallback for input regimes the device path does not cover."""
    b, s, _ = x.shape
    qkv = x @ Wqkv + bqkv
    qkv = qkv.reshape(b, s, H, 3 * HD).transpose(0, 2, 1, 3)
    q, k, v = np.split(qkv, 3, axis=-1)
    scores = np.einsum("bhqd,bhkd->bhqk", q, k)
    idx = np.arange(s)
    band = np.abs(idx[:, None] - idx[None, :]) <= 128
    pm = padding_mask != 0
    valid = band[None, None] & pm[:, None, None, :] & pm[:, None, :, None]
    scores = np.where(valid, scores, -np.inf) / np.sqrt(HD)
    scores = scores - scores.max(axis=-1, keepdims=True)
    with np.errstate(invalid="ignore", over="ignore"):
        e = np.exp(scores)
        attn = e / e.sum(axis=-1, keepdims=True)
    attn = np.nan_to_num(attn, nan=0.0)
    vals = np.einsum("bhqk,bhkd->bhqd", attn, v)
    vals = vals.transpose(0, 2, 1, 3).reshape(b, s, E)
    return (vals @ Wo + bo).astype(np.float32)


def kernel(x, padding_mask, Wqkv, bqkv, Wo, bo):
    global LAST_RESULTS
    x = np.ascontiguousarray(np.asarray(x, np.float32))
    Wqkv = np.asarray(Wqkv, np.float32)
    bqkv = np.asarray(bqkv, np.float32)
    Wo = np.asarray(Wo, np.float32)
    bo = np.asarray(bo, np.float32)
    pm = np.asarray(padding_mask)

    if np.any(bqkv != 0):
        # qkv bias is identically zero in the target problem; the device
        # program folds no qkv bias, so fall back rather than be wrong.
        return _numpy_reference(x, pm, Wqkv, bqkv, Wo, bo)

    if "nc" not in _CACHE:
        _CACHE["nc"] = _build_nc()
    nc = _CACHE["nc"]

    # band mask edge blocks [key p, {lower, upper}]
    j = np.arange(128)[:, None]
    i = np.arange(128)[None, :]
    tm = np.concatenate([(j <= i), (j >= i)], axis=1).astype(BF)

    in_maps = []
    for core in range(8):
        b, g = divmod(core, 4)
        # feature permutation for this head group: [q0|q1|k0|k1|v0|v1]
        h0, h1 = 2 * g, 2 * g + 1
        cols = []
        for kind in range(3):  # q, k, v
            for h in (h0, h1):
                base = h * 3 * HD + kind * HD
                cols.extend(range(base, base + HD))
        wq_g = Wqkv[:, cols]                                  # [512, 384]
        xt_b = np.ascontiguousarray(x[b].T)                   # [512, 2048]
        xt_cc = np.stack([xt_b[:, cc * 512:(cc + 1) * 512] for cc in range(4)])
        km = np.where(pm[b] != 0, 0.0, -1e5).astype(np.float32)
        in_maps.append({
            "xt": np.ascontiguousarray(xt_cc).astype(BF),
            "wq": np.ascontiguousarray(
                wq_g.reshape(4, 128, 384).transpose(1, 0, 2)).astype(BF),
            "wo": np.ascontiguousarray(Wo[g * 128:(g + 1) * 128, :]).astype(BF),
            "km": np.ascontiguousarray(km.reshape(NB, 128).T,
                                       dtype=np.float32),
            "tm": tm,
            "idin": np.eye(128, dtype=BF),
        })

    try:
        LAST_RESULTS = run_bass_kernel_spmd(nc, in_maps, core_ids=list(range(8)))
    except Exception:
        # transient device faults (e.g. NRT_EXEC_UNIT_UNRECOVERABLE) have been
        # observed to clear on the next attempt; retry once before giving up
        LAST_RESULTS = run_bass_kernel_spmd(nc, in_maps, core_ids=list(range(8)))
    res = LAST_RESULTS.results

    out = np.zeros((B, S, E), np.float32)
    for core in range(8):
        b = core // 4
        out[b] += np.asarray(res[core]["outt"], np.float32).T
    out += bo
    return out


# revision 12
# speedup vs baseline: 1.0391x; 1.0391x over previous
"""Banded (sliding-window) multi-head attention on 8 Trainium2 NeuronCores.

Problem: B=2, S=2048, D=512, H=8 heads (hd=64), window=256 (|i-j| <= 128),
  qkv = x @ Wqkv + bqkv           -> per-head q,k,v
  scores = (q k^T masked to band) / 8 ; softmax ; out = (attn v) @ Wo + bo

Sharding: core = (batch b in {0,1}) x (head-group g in {0..3}); each core
computes 2 heads over the full sequence of one batch element plus the o_proj
partial product for its heads' embed slice. The host sums the 4 partials per
batch and adds bo once.

Device-side layout (bf16 matmul operands, fp32 PSUM accumulation):
  - qkv projection computed TRANSPOSED: qkvT[fo, s], features permuted to
    [q0|q1|k0|k1|v0|v1] so Q^T/K^T/V^T per head live at partition offsets
    {0,64}.
  - scores key-major per 128-key block kb vs the 1-3 query blocks in band:
    ST[key, query] in PSUM -> ACT: P = exp(ST/8 + kmask[key]) -> DVE: the two
    128-col edge blocks *= triangular band mask (strided single op).
  - V^T re-transposed to natural [key, hd] via PE (bf16 transposes), with a
    ones column at 64/129 so denominators fall out of the AV matmul.
  - AV is FLIPPED vs v1: per query block qb, out[128q, 65] = sum_kb
    P[k, qb]^T @ [V|1]: 65-col matmuls (cheap in the col-based PE cost
    model), and the softmax denominator lands in PSUM as a *column*
    [128q, 1] -> strided reciprocal over 4 query blocks costs ~130ns,
    and no broadcast matmul is needed: normalization is a per-partition
    tensor_scalar multiply in SBUF bf16 (4x DVE mode).
  - valst_nat[q, d] is re-transposed per (h, qb) on PE into a per-chunk
    PSUM tile -> one copy -> d-major valstT for o_proj.
  - o_proj: outT[fo, s] partial = Wo_g^T @ valstT, written bf16 (host sums
    partials in fp32 and adds bo).
"""

import numpy as np
import ml_dtypes

import concourse.bass as bass  # noqa: F401
import concourse.mybir as mybir
import concourse.tile as tile
from concourse import bacc
from concourse.bass_utils import run_bass_kernel_spmd

B, S, DIN, E = 2, 2048, 512, 512
H, HD = 8, 64
NB = S // 128      # 16 key/query blocks of 128
NCHUNK = S // 512  # 4 query chunks of 512
F32 = mybir.dt.float32
F32R = mybir.dt.float32r
BF16 = mybir.dt.bfloat16
EXPF = mybir.ActivationFunctionType.Exp
IDENT = mybir.ActivationFunctionType.Identity
BF = ml_dtypes.bfloat16

_CACHE = {}
LAST_RESULTS = None  # BassKernelResults of the most recent run (for test.py)


def _build_nc():
    nc = bacc.Bacc(None, target_bir_lowering=False, debug=False)

    xt = nc.dram_tensor("xt", [4, DIN, 512], BF16, kind="ExternalInput")
    wq = nc.dram_tensor("wq", [128, 4, 384], BF16, kind="ExternalInput")
    wo = nc.dram_tensor("wo", [128, E], BF16, kind="ExternalInput")
    km = nc.dram_tensor("km", [128, NB], F32, kind="ExternalInput")
    tm = nc.dram_tensor("tm", [128, 256], BF16, kind="ExternalInput")
    idin = nc.dram_tensor("idin", [128, 128], BF16, kind="ExternalInput")
    outt = nc.dram_tensor("outt", [E, S], BF16, kind="ExternalOutput")

    with tile.TileContext(nc) as tc:
        with (
            tc.tile_pool(name="sb", bufs=1) as sb,
            tc.tile_pool(name="ps_qkv", bufs=2, space="PSUM") as ps_qkv,
            tc.tile_pool(name="ps_st", bufs=2, space="PSUM") as ps_st,
            tc.tile_pool(name="ps_av", bufs=2, space="PSUM") as ps_av,
            tc.tile_pool(name="ps_t", bufs=2, space="PSUM") as ps_t,
            tc.tile_pool(name="small", bufs=4) as small,
        ):
            xt_sb = sb.tile([128, 4, 4, 512], BF16)   # [p, kchunk, qchunk, q]
            wq_sb = sb.tile([128, 4, 384], BF16)      # [p, kchunk, fo]
            wo_sb = sb.tile([128, E], BF16)
            km_sb = sb.tile([128, NB], F32)
            tmE_sb = sb.tile([128, 2, 128], BF16)     # [lower | upper] edges
            qkvt = sb.tile([128, 2, S], BF16)         # fb0=Q, fb1=K
            vnat = sb.tile([128, NB, 130], BF16)      # [v0|1|v1|1] per key blk
            vln = sb.tile([128, 2, 4, 64], BF16)      # valst_nat per (h, qb)
            rc_sb = sb.tile([128, 2, 4], F32)         # 1/denom per (h, qb)
            valstT = sb.tile([128, S], BF16)          # d-major normalized AV
            outt_sb = sb.tile([128, 4, S], BF16)
            ident = sb.tile([128, 128], BF16)

            # weights/constants on the ACT HWDGE ring, xt on the SP ring;
            # both split by k-chunk so the first qkv group starts early
            for kc in range(4):
                nc.scalar.dma_start(out=wq_sb[:, kc, :], in_=wq[:, kc, :])
                nc.sync.dma_start(
                    out=xt_sb[:, kc, 0, :],
                    in_=xt[0, kc * 128:(kc + 1) * 128, :],
                )
            for cc in range(1, 4):
                nc.sync.dma_start(
                    out=xt_sb[:, :, cc, :],
                    in_=xt[cc].rearrange("(kc p) q -> p kc q", p=128),
                )
            nc.scalar.dma_start(out=km_sb, in_=km[:, :])
            nc.scalar.dma_start(
                out=tmE_sb, in_=tm.rearrange("p (a b) -> p a b", b=128))
            nc.sync.dma_start(out=ident, in_=idin[:, :])
            nc.sync.dma_start(out=wo_sb, in_=wo[:, :])

            # ones columns for the AV denominator
            nc.gpsimd.memset(vnat[:, :, 64:65], 1.0)
            nc.gpsimd.memset(vnat[:, :, 129:130], 1.0)

            # ---- q/k projection (transposed): qkT = Wg^T @ x[b]^T ----
            def qkv_tile(cc, fb):
                ps = ps_qkv.tile([128, 512], F32, tag="qkv", name="ps")
                for kc in range(4):
                    nc.tensor.matmul(
                        ps,
                        wq_sb[:, kc, fb * 128:(fb + 1) * 128],
                        xt_sb[:, kc, cc, :],
                        start=(kc == 0),
                        stop=(kc == 3),
                    )
                dst = qkvt[:, fb, cc * 512:(cc + 1) * 512]
                if (cc, fb) in ((0, 0), (2, 0)):
                    nc.scalar.activation(out=dst, in_=ps, func=IDENT)
                else:
                    nc.vector.tensor_copy(dst, ps)

            # ---- V computed directly NATURAL: V[q, hd] = x @ Wv per query
            # block (no transpose round-trip); ones cols at 64/129 ----
            def pst_group(cc):
                kb0 = 4 * cc
                vps = ps_t.tile([128, 4, 128], F32, tag="t", name="vps")
                for j in range(4):
                    for kc in range(4):
                        nc.tensor.matmul(
                            vps[:, j, :],
                            xt_sb[:, kc, cc, (kb0 + j - 4 * cc) * 128:
                                  (kb0 + j - 4 * cc + 1) * 128],
                            wq_sb[:, kc, 256:384],
                            start=(kc == 0),
                            stop=(kc == 3),
                        )
                nc.vector.tensor_copy(
                    vnat[:, kb0:kb0 + 4, :]
                    .rearrange("p k (g c) -> p k g c", c=65)[:, :, :, 0:64],
                    vps.rearrange("p k (g c) -> p k g c", c=64),
                )

            # ---- attention ----
            p_sb = [sb.tile([128, NB, 384], BF16, name=f"p{h}")
                    for h in range(2)]

            def scores_block(h, kb):
                hp = 64 * h
                ws, we = max(0, kb - 1), min(NB - 1, kb + 1)
                nq = (we - ws + 1) * 128
                st = ps_st.tile([128, 384], F32, tag="st", name="st")
                nc.tensor.matmul(
                    st[:, :nq],
                    qkvt[hp:hp + 64, 1, kb * 128:(kb + 1) * 128],
                    qkvt[hp:hp + 64, 0, ws * 128:(we + 1) * 128],
                    start=True,
                    stop=True,
                )
                nc.scalar.activation(
                    out=p_sb[h][:, kb, 0:nq],
                    in_=st[:, :nq],
                    func=EXPF,
                    bias=km_sb[:, kb:kb + 1],
                    scale=0.125,
                )
                # band-mask only the edge blocks (lower on the kb-1 block,
                # upper on the kb+1 block); the center block is all-valid.
                # Pool takes a share (SBUF-only op) to unload DVE.
                eng = nc.gpsimd if (2 * kb + h) % 8 < 3 else nc.vector
                pv = p_sb[h][:, kb, :].rearrange("p (a b) -> p a b", b=128)
                if kb == 0:
                    eng.tensor_mul(
                        pv[:, 1, :], pv[:, 1, :], tmE_sb[:, 1, :])
                elif kb == NB - 1:
                    eng.tensor_mul(
                        pv[:, 0, :], pv[:, 0, :], tmE_sb[:, 0, :])
                else:
                    eng.tensor_mul(
                        pv[:, 0::2, :], pv[:, 0::2, :], tmE_sb)

            # flipped AV: per query block, out[128q, 65] accumulates
            # P[k, qb]^T @ [V|1] over the 2-3 contributing key blocks
            av_tiles = {}  # h -> live av PSUM tile for the current chunk

            def av_qb(h, qb):
                if qb % 4 == 0 or h not in av_tiles:
                    av_tiles[h] = ps_av.tile(
                        [128, 4, 65], F32, tag="av", name="av")
                av = av_tiles[h]
                j = qb % 4
                kbs = [kb for kb in (qb - 1, qb, qb + 1) if 0 <= kb <= NB - 1]
                for i, kb in enumerate(kbs):
                    ws = max(0, kb - 1)
                    nc.tensor.matmul(
                        av[:, j, :],
                        p_sb[h][:, kb, (qb - ws) * 128:(qb - ws + 1) * 128],
                        vnat[:, kb, 65 * h:65 * h + 65],
                        start=(i == 0),
                        stop=(i == len(kbs) - 1),
                    )

            def chunk_finish(c):
                for h in range(2):
                    av = av_tiles.pop(h)
                    with nc.allow_low_precision("f32r softmax denom recip"):
                        nc.vector.reciprocal(rc_sb[:, h, :], av[:, :, 64])
                    nc.vector.tensor_copy(vln[:, h, :, :], av[:, :, 0:64])
                    for j in range(4):
                        nc.gpsimd.tensor_scalar_mul(
                            out=vln[:, h, j, :],
                            in0=vln[:, h, j, :],
                            scalar1=rc_sb[:, h, j:j + 1],
                        )
                # transpose normalized [q, d] -> d-major [2h*64, 512q]
                vt = ps_t.tile([128, 512], BF16, tag="t", name="vt")
                for h in range(2):
                    for j in range(4):
                        nc.tensor.transpose(
                            vt[64 * h:64 * h + 64, j * 128:(j + 1) * 128],
                            vln[:, h, j, :],
                            ident,
                        )
                nc.vector.tensor_copy(valstT[:, c * 512:(c + 1) * 512], vt)
                for fo in range(4):
                    po = ps_qkv.tile([128, 512], F32, tag="qkv", name="po")
                    nc.tensor.matmul(
                        po,
                        wo_sb[:, fo * 128:(fo + 1) * 128],
                        valstT[:, c * 512:(c + 1) * 512],
                        start=True,
                        stop=True,
                    )
                    dst = outt_sb[:, fo, c * 512:(c + 1) * 512]
                    if fo % 2 == 0:
                        nc.scalar.activation(out=dst, in_=po, func=IDENT)
                    else:
                        nc.vector.tensor_copy(dst, po)
                    nc.sync.dma_start(
                        out=outt[fo * 128:(fo + 1) * 128,
                                 c * 512:(c + 1) * 512],
                        in_=dst,
                    )

            # software pipeline: chunk-0 qkv + V up front, then per key
            # block: late qkv tiles for the next chunk, scores, the AV for
            # the query block completed by this kb, and the chunk epilogue
            # (normalize/transpose/o_proj) at chunk boundaries.
            for fb in range(2):
                qkv_tile(0, fb)
            pst_group(0)
            for kb in range(NB):
                cc = kb // 4
                if cc < 3:
                    if kb % 4 == 1:
                        qkv_tile(cc + 1, 0)
                    elif kb % 4 == 2:
                        qkv_tile(cc + 1, 1)
                    elif kb % 4 == 3:
                        pst_group(cc + 1)
                for h in range(2):
                    scores_block(h, kb)
                if kb >= 1:
                    for h in range(2):
                        av_qb(h, kb - 1)
                    if kb % 4 == 0:
                        chunk_finish(kb // 4 - 1)
            for h in range(2):
                av_qb(h, NB - 1)
            chunk_finish(NCHUNK - 1)

    nc.finalize()
    return nc


def _numpy_reference(x, padding_mask, Wqkv, bqkv, Wo, bo):
    """Fallback for input regimes the device path does not cover."""
    b, s, _ = x.shape
    qkv = x @ Wqkv + bqkv
    qkv = qkv.reshape(b, s, H, 3 * HD).transpose(0, 2, 1, 3)
    q, k, v = np.split(qkv, 3, axis=-1)
    scores = np.einsum("bhqd,bhkd->bhqk", q, k)
    idx = np.arange(s)
    band = np.abs(idx[:, None] - idx[None, :]) <= 128
    pm = padding_mask != 0
    valid = band[None, None] & pm[:, None, None, :] & pm[:, None, :, None]
    scores = np.where(valid, scores, -np.inf) / np.sqrt(HD)
    scores = scores - scores.max(axis=-1, keepdims=True)
    with np.errstate(invalid="ignore", over="ignore"):
        e = np.exp(scores)
        attn = e / e.sum(axis=-1, keepdims=True)
    attn = np.nan_to_num(attn, nan=0.0)
    vals = np.einsum("bhqk,bhkd->bhqd", attn, v)
    vals = vals.transpose(0, 2, 1, 3).reshape(b, s, E)
    return (vals @ Wo + bo).astype(np.float32)


def kernel(x, padding_mask, Wqkv, bqkv, Wo, bo):
    global LAST_RESULTS
    x = np.ascontiguousarray(np.asarray(x, np.float32))
    Wqkv = np.asarray(Wqkv, np.float32)
    bqkv = np.asarray(bqkv, np.float32)
    Wo = np.asarray(Wo, np.float32)
    bo = np.asarray(bo, np.float32)
    pm = np.asarray(padding_mask)

    if np.any(bqkv != 0):
        # qkv bias is identically zero in the target problem; the device
        # program folds no qkv bias, so fall back rather than be wrong.
        return _numpy_reference(x, pm, Wqkv, bqkv, Wo, bo)

    if "nc" not in _CACHE:
        _CACHE["nc"] = _build_nc()
    nc = _CACHE["nc"]

    # band mask edge blocks [key p, {lower, upper}]
    j = np.arange(128)[:, None]
    i = np.arange(128)[None, :]
    tm = np.concatenate([(j <= i), (j >= i)], axis=1).astype(BF)

    in_maps = []
    for core in range(8):
        b, g = divmod(core, 4)
        # feature permutation for this head group: [q0|q1|k0|k1|v0|v1]
        h0, h1 = 2 * g, 2 * g + 1
        cols = []
        for kind in range(3):  # q, k, v
            for h in (h0, h1):
                base = h * 3 * HD + kind * HD
                cols.extend(range(base, base + HD))
        wq_g = Wqkv[:, cols]                                  # [512, 384]
        xt_b = np.ascontiguousarray(x[b].T)                   # [512, 2048]
        xt_cc = np.stack([xt_b[:, cc * 512:(cc + 1) * 512] for cc in range(4)])
        km = np.where(pm[b] != 0, 0.0, -1e5).astype(np.float32)
        in_maps.append({
            "xt": np.ascontiguousarray(xt_cc).astype(BF),
            "wq": np.ascontiguousarray(
                wq_g.reshape(4, 128, 384).transpose(1, 0, 2)).astype(BF),
            "wo": np.ascontiguousarray(Wo[g * 128:(g + 1) * 128, :]).astype(BF),
            "km": np.ascontiguousarray(km.reshape(NB, 128).T,
                                       dtype=np.float32),
            "tm": tm,
            "idin": np.eye(128, dtype=BF),
        })

    try:
        LAST_RESULTS = run_bass_kernel_spmd(nc, in_maps, core_ids=list(range(8)))
    except Exception:
        # transient device faults (e.g. NRT_EXEC_UNIT_UNRECOVERABLE) have been
        # observed to clear on the next attempt; retry once before giving up
        LAST_RESULTS = run_bass_kernel_spmd(nc, in_maps, core_ids=list(range(8)))
    res = LAST_RESULTS.results

    out = np.zeros((B, S, E), np.float32)
    for core in range(8):
        b = core // 4
        out[b] += np.asarray(res[core]["outt"], np.float32).T
    out += bo
    return out


# revision 16
# speedup vs baseline: 1.0999x; 1.0585x over previous
"""Banded (sliding-window) multi-head attention on 8 Trainium2 NeuronCores.

Problem: B=2, S=2048, D=512, H=8 heads (hd=64), window=256 (|i-j| <= 128),
  qkv = x @ Wqkv + bqkv           -> per-head q,k,v
  scores = (q k^T masked to band) / 8 ; softmax ; out = (attn v) @ Wo + bo

Sharding: core = (batch b in {0,1}) x (head-group g in {0..3}); each core
computes 2 heads over the full sequence of one batch element plus the o_proj
partial product for its heads' embed slice. The host sums the 4 partials per
batch and adds bo once.

Device-side layout (bf16 matmul operands, fp32 PSUM accumulation):
  - qkv projection computed TRANSPOSED: qkvT[fo, s], features permuted to
    [q0|q1|k0|k1|v0|v1] so Q^T/K^T/V^T per head live at partition offsets
    {0,64}.
  - scores key-major per 128-key block kb vs the 1-3 query blocks in band:
    ST[key, query] in PSUM -> ACT: P = exp(ST/8 + kmask[key]) -> DVE: the two
    128-col edge blocks *= triangular band mask (strided single op).
  - V^T re-transposed to natural [key, hd] via PE (bf16 transposes), with a
    ones column at 64/129 so denominators fall out of the AV matmul.
  - AV is FLIPPED vs v1: per query block qb, out[128q, 65] = sum_kb
    P[k, qb]^T @ [V|1]: 65-col matmuls (cheap in the col-based PE cost
    model), and the softmax denominator lands in PSUM as a *column*
    [128q, 1] -> strided reciprocal over 4 query blocks costs ~130ns,
    and no broadcast matmul is needed: normalization is a per-partition
    tensor_scalar multiply in SBUF bf16 (4x DVE mode).
  - valst_nat[q, d] is re-transposed per (h, qb) on PE into a per-chunk
    PSUM tile -> one copy -> d-major valstT for o_proj.
  - o_proj: outT[fo, s] partial = Wo_g^T @ valstT, written bf16 (host sums
    partials in fp32 and adds bo).
"""

import numpy as np
import ml_dtypes

import concourse.bass as bass  # noqa: F401
import concourse.mybir as mybir
import concourse.tile as tile
from concourse import bacc
from concourse.bass_utils import run_bass_kernel_spmd

B, S, DIN, E = 2, 2048, 512, 512
H, HD = 8, 64
NB = S // 128      # 16 key/query blocks of 128
NCHUNK = S // 512  # 4 query chunks of 512
F32 = mybir.dt.float32
F32R = mybir.dt.float32r
BF16 = mybir.dt.bfloat16
EXPF = mybir.ActivationFunctionType.Exp
IDENT = mybir.ActivationFunctionType.Identity
BF = ml_dtypes.bfloat16

_CACHE = {}
LAST_RESULTS = None  # BassKernelResults of the most recent run (for test.py)


def _build_nc():
    nc = bacc.Bacc(None, target_bir_lowering=False, debug=False)

    xt = nc.dram_tensor("xt", [4, DIN, 512], BF16, kind="ExternalInput")
    wq = nc.dram_tensor("wq", [128, 4, 384], BF16, kind="ExternalInput")
    wo = nc.dram_tensor("wo", [128, E], BF16, kind="ExternalInput")
    km = nc.dram_tensor("km", [128, NB], F32, kind="ExternalInput")
    tm = nc.dram_tensor("tm", [128, 256], BF16, kind="ExternalInput")
    idin = nc.dram_tensor("idin", [128, 128], BF16, kind="ExternalInput")
    outt = nc.dram_tensor("outt", [E, S], BF16, kind="ExternalOutput")

    with tile.TileContext(nc) as tc:
        with (
            tc.tile_pool(name="sb", bufs=1) as sb,
            tc.tile_pool(name="ps_qkv", bufs=2, space="PSUM") as ps_qkv,
            tc.tile_pool(name="ps_st", bufs=2, space="PSUM") as ps_st,
            tc.tile_pool(name="ps_av", bufs=2, space="PSUM") as ps_av,
            tc.tile_pool(name="ps_t", bufs=2, space="PSUM") as ps_t,
            tc.tile_pool(name="small", bufs=4) as small,
        ):
            xt_sb = sb.tile([128, 4, 4, 512], BF16)   # [p, kchunk, qchunk, q]
            wq_sb = sb.tile([128, 4, 384], BF16)      # [p, kchunk, fo]
            wo_sb = sb.tile([128, E], BF16)
            km_sb = sb.tile([128, NB], F32)
            tmE_sb = sb.tile([128, 2, 128], BF16)     # [lower | upper] edges
            qkvt = sb.tile([128, 2, S], BF16)         # fb0=Q, fb1=K
            vnat = sb.tile([128, NB, 130], BF16)      # [v0|1|v1|1] per key blk
            vln = sb.tile([128, 2, 4, 64], BF16)      # valst_nat per (h, qb)
            rc_sb = sb.tile([128, 2, 4], F32)         # 1/denom per (h, qb)
            valstT = sb.tile([128, S], BF16)          # d-major normalized AV
            outt_sb = sb.tile([128, 4, S], BF16)
            ident = sb.tile([128, 128], BF16)

            # weights/constants on the ACT HWDGE ring, xt on the SP ring;
            # both split by k-chunk so the first qkv group starts early
            for kc in range(4):
                nc.scalar.dma_start(out=wq_sb[:, kc, :], in_=wq[:, kc, :])
                nc.sync.dma_start(
                    out=xt_sb[:, kc, 0, :],
                    in_=xt[0, kc * 128:(kc + 1) * 128, :],
                )
            for cc in range(1, 4):
                nc.sync.dma_start(
                    out=xt_sb[:, :, cc, :],
                    in_=xt[cc].rearrange("(kc p) q -> p kc q", p=128),
                )
            nc.scalar.dma_start(out=km_sb, in_=km[:, :])
            nc.scalar.dma_start(
                out=tmE_sb, in_=tm.rearrange("p (a b) -> p a b", b=128))
            nc.sync.dma_start(out=ident, in_=idin[:, :])
            nc.sync.dma_start(out=wo_sb, in_=wo[:, :])

            # ones columns for the AV denominator
            nc.gpsimd.memset(vnat[:, :, 64:65], 1.0)
            nc.gpsimd.memset(vnat[:, :, 129:130], 1.0)

            # ---- q/k projection (transposed): qkT = Wg^T @ x[b]^T ----
            def qkv_tile(cc, fb):
                ps = ps_qkv.tile([128, 512], F32, tag="qkv", name="ps")
                for kc in range(4):
                    nc.tensor.matmul(
                        ps,
                        wq_sb[:, kc, fb * 128:(fb + 1) * 128],
                        xt_sb[:, kc, cc, :],
                        start=(kc == 0),
                        stop=(kc == 3),
                    )
                dst = qkvt[:, fb, cc * 512:(cc + 1) * 512]
                if (cc, fb) in ((0, 0), (2, 0)):
                    nc.scalar.activation(out=dst, in_=ps, func=IDENT)
                else:
                    nc.vector.tensor_copy(dst, ps)

            # ---- V computed directly NATURAL: V[q, hd] = x @ Wv per query
            # block (no transpose round-trip); ones cols at 64/129 ----
            def pst_group(cc):
                kb0 = 4 * cc
                vps = ps_t.tile([128, 4, 128], F32, tag="t", name="vps")
                for j in range(4):
                    for kc in range(4):
                        nc.tensor.matmul(
                            vps[:, j, :],
                            xt_sb[:, kc, cc, (kb0 + j - 4 * cc) * 128:
                                  (kb0 + j - 4 * cc + 1) * 128],
                            wq_sb[:, kc, 256:384],
                            start=(kc == 0),
                            stop=(kc == 3),
                        )
                nc.vector.tensor_copy(
                    vnat[:, kb0:kb0 + 4, :]
                    .rearrange("p k (g c) -> p k g c", c=65)[:, :, :, 0:64],
                    vps.rearrange("p k (g c) -> p k g c", c=64),
                )

            # ---- attention ----
            p_sb = [sb.tile([128, NB, 384], BF16, name=f"p{h}")
                    for h in range(2)]

            def scores_block(h, kb):
                hp = 64 * h
                ws, we = max(0, kb - 1), min(NB - 1, kb + 1)
                nq = (we - ws + 1) * 128
                st = ps_st.tile([128, 384], F32, tag="st", name="st")
                nc.tensor.matmul(
                    st[:, :nq],
                    qkvt[hp:hp + 64, 1, kb * 128:(kb + 1) * 128],
                    qkvt[hp:hp + 64, 0, ws * 128:(we + 1) * 128],
                    start=True,
                    stop=True,
                )
                nc.scalar.activation(
                    out=p_sb[h][:, kb, 0:nq],
                    in_=st[:, :nq],
                    func=EXPF,
                    bias=km_sb[:, kb:kb + 1],
                    scale=0.125,
                )
                # band-mask only the edge blocks (lower on the kb-1 block,
                # upper on the kb+1 block); the center block is all-valid.
                # Pool takes a share (SBUF-only op) to unload DVE.
                eng = nc.gpsimd if h == 0 else nc.vector
                pv = p_sb[h][:, kb, :].rearrange("p (a b) -> p a b", b=128)
                if kb == 0:
                    eng.tensor_mul(
                        pv[:, 1, :], pv[:, 1, :], tmE_sb[:, 1, :])
                elif kb == NB - 1:
                    eng.tensor_mul(
                        pv[:, 0, :], pv[:, 0, :], tmE_sb[:, 0, :])
                else:
                    eng.tensor_mul(
                        pv[:, 0::2, :], pv[:, 0::2, :], tmE_sb)

            # flipped AV: per query block, out[128q, 65] accumulates
            # P[k, qb]^T @ [V|1] over the 2-3 contributing key blocks
            av_tiles = {}  # h -> live av PSUM tile for the current chunk

            def av_qb(h, qb):
                if qb % 4 == 0 or h not in av_tiles:
                    av_tiles[h] = ps_av.tile(
                        [128, 4, 65], F32, tag="av", name="av")
                av = av_tiles[h]
                j = qb % 4
                kbs = [kb for kb in (qb - 1, qb, qb + 1) if 0 <= kb <= NB - 1]
                for i, kb in enumerate(kbs):
                    ws = max(0, kb - 1)
                    nc.tensor.matmul(
                        av[:, j, :],
                        p_sb[h][:, kb, (qb - ws) * 128:(qb - ws + 1) * 128],
                        vnat[:, kb, 65 * h:65 * h + 65],
                        start=(i == 0),
                        stop=(i == len(kbs) - 1),
                    )

            def chunk_norm(c):
                # stage 1 of the chunk epilogue: denominators + normalize
                for h in range(2):
                    av = av_tiles.pop(h)
                    with nc.allow_low_precision("f32r softmax denom recip"):
                        nc.vector.reciprocal(rc_sb[:, h, :], av[:, :, 64])
                    nc.vector.tensor_copy(vln[:, h, :, :], av[:, :, 0:64])
                    for j in range(4):
                        nc.vector.tensor_scalar_mul(
                            out=vln[:, h, j, :],
                            in0=vln[:, h, j, :],
                            scalar1=rc_sb[:, h, j:j + 1],
                        )

            def chunk_oproj(c):
                # stage 2: transpose [q, d] -> d-major [2h*64, 512q], o_proj
                vt = ps_t.tile([128, 512], BF16, tag="t", name="vt")
                for h in range(2):
                    for j in range(4):
                        nc.tensor.transpose(
                            vt[64 * h:64 * h + 64, j * 128:(j + 1) * 128],
                            vln[:, h, j, :],
                            ident,
                        )
                nc.vector.tensor_copy(valstT[:, c * 512:(c + 1) * 512], vt)
                for fo in range(4):
                    po = ps_qkv.tile([128, 512], F32, tag="qkv", name="po")
                    nc.tensor.matmul(
                        po,
                        wo_sb[:, fo * 128:(fo + 1) * 128],
                        valstT[:, c * 512:(c + 1) * 512],
                        start=True,
                        stop=True,
                    )
                    dst = outt_sb[:, fo, c * 512:(c + 1) * 512]
                    if fo % 2 == 0:
                        nc.scalar.activation(out=dst, in_=po, func=IDENT)
                    else:
                        nc.vector.tensor_copy(dst, po)
                    nc.sync.dma_start(
                        out=outt[fo * 128:(fo + 1) * 128,
                                 c * 512:(c + 1) * 512],
                        in_=dst,
                    )

            # software pipeline: chunk-0 qkv + V up front, then per key
            # block: late qkv tiles for the next chunk, scores, the AV for
            # the query block completed by this kb, and the chunk epilogue
            # (normalize/transpose/o_proj) at chunk boundaries.
            for fb in range(2):
                qkv_tile(0, fb)
            pst_group(0)
            for kb in range(NB):
                cc = kb // 4
                if cc < 3:
                    if kb % 4 == 1:
                        qkv_tile(cc + 1, 0)
                    elif kb % 4 == 2:
                        qkv_tile(cc + 1, 1)
                    elif kb % 4 == 3:
                        pst_group(cc + 1)
                for h in range(2):
                    scores_block(h, kb)
                if kb >= 1:
                    for h in range(2):
                        av_qb(h, kb - 1)
                    if kb % 4 == 0:
                        chunk_norm(kb // 4 - 1)
                    elif kb % 4 == 1 and kb >= 5:
                        chunk_oproj(kb // 4 - 1)
            for h in range(2):
                av_qb(h, NB - 1)
            chunk_norm(NCHUNK - 1)
            chunk_oproj(NCHUNK - 1)

    nc.finalize()
    return nc


def _numpy_reference(x, padding_mask, Wqkv, bqkv, Wo, bo):
    """Fallback for input regimes the device path does not cover."""
    b, s, _ = x.shape
    qkv = x @ Wqkv + bqkv
    qkv = qkv.reshape(b, s, H, 3 * HD).transpose(0, 2, 1, 3)
    q, k, v = np.split(qkv, 3, axis=-1)
    scores = np.einsum("bhqd,bhkd->bhqk", q, k)
    idx = np.arange(s)
    band = np.abs(idx[:, None] - idx[None, :]) <= 128
    pm = padding_mask != 0
    valid = band[None, None] & pm[:, None, None, :] & pm[:, None, :, None]
    scores = np.where(valid, scores, -np.inf) / np.sqrt(HD)
    scores = scores - scores.max(axis=-1, keepdims=True)
    with np.errstate(invalid="ignore", over="ignore"):
        e = np.exp(scores)
        attn = e / e.sum(axis=-1, keepdims=True)
    attn = np.nan_to_num(attn, nan=0.0)
    vals = np.einsum("bhqk,bhkd->bhqd", attn, v)
    vals = vals.transpose(0, 2, 1, 3).reshape(b, s, E)
    return (vals @ Wo + bo).astype(np.float32)


def kernel(x, padding_mask, Wqkv, bqkv, Wo, bo):
    global LAST_RESULTS
    x = np.ascontiguousarray(np.asarray(x, np.float32))
    Wqkv = np.asarray(Wqkv, np.float32)
    bqkv = np.asarray(bqkv, np.float32)
    Wo = np.asarray(Wo, np.float32)
    bo = np.asarray(bo, np.float32)
    pm = np.asarray(padding_mask)

    if np.any(bqkv != 0):
        # qkv bias is identically zero in the target problem; the device
        # program folds no qkv bias, so fall back rather than be wrong.
        return _numpy_reference(x, pm, Wqkv, bqkv, Wo, bo)

    if "nc" not in _CACHE:
        _CACHE["nc"] = _build_nc()
    nc = _CACHE["nc"]

    # band mask edge blocks [key p, {lower, upper}]
    j = np.arange(128)[:, None]
    i = np.arange(128)[None, :]
    tm = np.concatenate([(j <= i), (j >= i)], axis=1).astype(BF)

    in_maps = []
    for core in range(8):
        b, g = divmod(core, 4)
        # feature permutation for this head group: [q0|q1|k0|k1|v0|v1]
        h0, h1 = 2 * g, 2 * g + 1
        cols = []
        for kind in range(3):  # q, k, v
            for h in (h0, h1):
                base = h * 3 * HD + kind * HD
                cols.extend(range(base, base + HD))
        wq_g = Wqkv[:, cols]                                  # [512, 384]
        xt_b = np.ascontiguousarray(x[b].T)                   # [512, 2048]
        xt_cc = np.stack([xt_b[:, cc * 512:(cc + 1) * 512] for cc in range(4)])
        km = np.where(pm[b] != 0, 0.0, -1e5).astype(np.float32)
        in_maps.append({
            "xt": np.ascontiguousarray(xt_cc).astype(BF),
            "wq": np.ascontiguousarray(
                wq_g.reshape(4, 128, 384).transpose(1, 0, 2)).astype(BF),
            "wo": np.ascontiguousarray(Wo[g * 128:(g + 1) * 128, :]).astype(BF),
            "km": np.ascontiguousarray(km.reshape(NB, 128).T,
                                       dtype=np.float32),
            "tm": tm,
            "idin": np.eye(128, dtype=BF),
        })

    try:
        LAST_RESULTS = run_bass_kernel_spmd(nc, in_maps, core_ids=list(range(8)))
    except Exception:
        # transient device faults (e.g. NRT_EXEC_UNIT_UNRECOVERABLE) have been
        # observed to clear on the next attempt; retry once before giving up
        LAST_RESULTS = run_bass_kernel_spmd(nc, in_maps, core_ids=list(range(8)))
    res = LAST_RESULTS.results

    out = np.zeros((B, S, E), np.float32)
    for core in range(8):
        b = core // 4
        out[b] += np.asarray(res[core]["outt"], np.float32).T
    out += bo
    return out
